# revision 28
# baseline (speedup 1.0000x reference)
"""Trainium2 Bass kernel: causal multi-head attention (dense transformer block).

Reference semantics (quirk: scores = K @ Q^T, scaled by C**-0.5):
    k = x @ Wk ; q = x @ Wq ; v = x @ Wv          (per-head split, H=16, D=64)
    wei[i, j] = (k_i . q_j) * C**-0.5,  masked j <= i, softmax over j
    out = (wei @ v) @ Wproj + bproj

Sharding (8 cores): data-parallel over batch (4) x tensor-parallel over
head-halves (2).  Core c handles batch c//2, heads [8*(c%2), 8*(c%2)+8).
Each core returns the transposed partial projection output in bf16;
the host sums the two partials per batch and transposes back in fp32.

v2 design (fused single-stream pipeline):
  - Whole kernel is ONE stream: QKV projection chunks, attention spans and
    the output projection are interleaved so PE / ACT / DVE / GPSIMD all
    stay busy.  QKV chunk c+1 and output-proj span s-1 are emitted as
    "filler" thunks woven between attention units of span s.
  - All activations bf16 (qrT/kcT/v/wei/attnT, weights cast once per rep);
    matmuls run bf16 at full PE rate; x is transposed on PE in fp32 and
    cast to bf16 in the PSUM->SBUF copy.
  - exp is split between ScalarE (activation Exp) and a custom 8-stage DVE
    op EXPQ: exp(s*SCALE) ~= ((1 + s(c1 + s(c2 + s*c3)))^2)^2, a degree-3
    minimax fit squared twice (always positive, exact at 0, ~1e-3 rel err
    over the +-64 score range).  Ratio set by EXP_PATTERN.
  - softmax denominator rides as a ones-column in V (AV output row 64);
    1/d via DVE reciprocal_approx_fast, broadcast across partitions by a
    DRAM-bounce DMA, folded in with a DVE multiply.
  - PSUM budget (8 banks): shared "blk" pool (scores / QKV / transposes /
    out-proj, 2 x [128,2,512] slots = 4 banks) + pav pool (AV accumulate,
    2 x [65,2,512] = 4 banks).
"""

import numpy as np

import concourse.bass as bass  # noqa: F401
import concourse.tile as tile
from concourse import bacc, mybir
from concourse.bass_utils import run_bass_kernel_spmd
from concourse.masks import make_identity

F32 = mybir.dt.float32
BF16 = mybir.dt.bfloat16

B, T_FULL, C = 4, 2048, 1024
H, NCORES, P, NS, D = 16, 8, 128, 512, 64
CH = C // 2            # per-core channels (8 heads)
HPC = CH // D          # heads per core
MT = CH // P           # head-pair tiles
KT = C // P            # k-tiles of the model dim
SCALE = 1.0 / float(np.sqrt(C))
DEPTH = 2

# exp engine per attention unit: 'A' = ScalarE, 'D' = custom DVE poly.
EXP_PATTERN = "AAD"


def _fit_expq(smax=64.0):
    """Fit p(s) = 1 + c1 s + c2 s^2 + c3 s^3 so that p^4 ~= exp(SCALE*s)
    on [-smax, smax], minimizing max relative error (Lawson iteration)."""
    s = np.linspace(-smax, smax, 4001)
    t = np.exp(SCALE * s / 4.0)
    A = np.stack([s, s * s, s**3], 1)
    w = np.ones_like(s)
    c = np.zeros(3)
    for _ in range(80):
        Wt = (w / t)[:, None]
        c, *_ = np.linalg.lstsq(A * Wt, (t - 1.0) * Wt[:, 0], rcond=None)
        r = np.abs((1.0 + A @ c) / t - 1.0)
        w = w * np.maximum(r, 1e-9) ** 0.7
        w /= w.mean()
    g = (1.0 + A @ c) ** 4
    rel = np.abs(g / np.exp(SCALE * s) - 1.0)
    return [float(v) for v in c], float(rel.max())


_EXPQ_C, _EXPQ_MAXERR = _fit_expq()
_EXPQ_OP = None


def _get_expq():
    global _EXPQ_OP
    if _EXPQ_OP is None:
        from concourse.dve_spec import Spec, Src0, C0, C1, C2, One, sq, lower
        from concourse.dve_uop import DveOpSpec
        from concourse.dve_ops import DveOp, OPS

        name = "EXPQ_MHA_V1"
        body = sq(sq(One + Src0 * (C0 + Src0 * (C1 + Src0 * C2))))

        def _ref(in0, in1, s0, s1, imm2):
            p = 1.0 + in0 * (s0 + in0 * (s1 + in0 * imm2))
            p = p * p
            return p * p

        spec = Spec(body=body, reference=_ref)
        shas = {
            ver: DveOpSpec(
                name=name, opcode=0, uops=lower(spec, ver=ver), rd1_en=False
            ).sha(ver)
            for ver in ("v3", "v4")
        }
        op = DveOp(name, spec, subdim=False, uops_sha=shas)
        for i, o in enumerate(OPS):
            if o.name == name:
                OPS[i] = op
                break
        else:
            OPS.append(op)
        from concourse import dve_ops as _dops

        _dops.CUSTOM_DVE_SPECS[name] = spec
        _dops._SUB_OPCODE_FOR_NAME[name] = _dops._CUSTOM_DVE_ROW_BASE + next(
            i for i, o in enumerate(OPS) if o.name == name
        )
        assert max(_dops._SUB_OPCODE_FOR_NAME.values()) < 0x20
        _EXPQ_OP = op
    return _EXPQ_OP


def _emit_prelude(ctx, tc, aps, T):
    """Once-per-kernel setup: constants, weight loads + bf16 casts, and the
    persistent SBUF tiles.  Kept outside the reps loop."""
    nc = tc.nc
    wr, wc, wv, wp, bias_d = (aps[k] for k in ("wr", "wc", "wv", "wp", "bias"))
    TTL = T // P

    consts = ctx.enter_context(tc.tile_pool(name="consts", bufs=1))
    ident = consts.tile([P, P], F32)
    make_identity(nc, ident)
    onesf = consts.tile([P, P], F32)
    nc.vector.memset(onesf, 1.0)
    bias_sb = consts.tile([P, C // P], F32)
    nc.sync.dma_start(out=bias_sb, in_=bias_d)

    acts = ctx.enter_context(tc.tile_pool(name="acts", bufs=1))
    qrT = acts.tile([P, MT, T], BF16)      # row operand (x@Wk), head-transposed
    kcT = acts.tile([P, MT, T], BF16)      # col operand (x@Wq), head-transposed
    v_sb = acts.tile([P, TTL, HPC, D + 1], BF16)  # V + ones column per head
    attnT = acts.tile([P, MT, T], BF16)
    wqkv = acts.tile([P, 3, KT, CH], BF16)
    wp_b = acts.tile([P, MT, C], BF16)

    nc.vector.tensor_copy(
        v_sb[:, :, :, D : D + 1],
        onesf[:, 0 : TTL * HPC].rearrange("p (a b c) -> p a b c", a=TTL, b=HPC, c=1),
    )

    with tc.tile_pool(name="stage", bufs=2) as stage:
        for wi, wd in enumerate((wr, wc, wv)):
            wre = wd.rearrange("(kt p) ch -> p kt ch", p=P)
            for hh in range(2):
                st = stage.tile([P, KT // 2, CH], F32, tag="st")
                nc.gpsimd.dma_start(out=st, in_=wre[:, 4 * hh : 4 * hh + 4, :])
                nc.scalar.copy(wqkv[:, wi, 4 * hh : 4 * hh + 4, :], st)
        wpre = wp.rearrange("(cp p) c -> p cp c", p=P)
        for cp in range(MT):
            st = stage.tile([P, C], F32, tag="st")
            nc.gpsimd.dma_start(out=st, in_=wpre[:, cp, :])
            nc.scalar.copy(wp_b[:, cp, :], st)

    return dict(
        ident=ident, bias_sb=bias_sb, qrT=qrT, kcT=kcT, v_sb=v_sb,
        attnT=attnT, wqkv=wqkv, wp_b=wp_b,
    )


def _emit(ctx, tc, aps, T, pre, phases=(1, 2, 3)):
    nc = tc.nc
    x, outT = aps["x"], aps["outT"]
    ident, bias_sb = pre["ident"], pre["bias_sb"]
    qrT, kcT, v_sb, attnT = pre["qrT"], pre["kcT"], pre["v_sb"], pre["attnT"]
    wqkv, wp_b = pre["wqkv"], pre["wp_b"]
    TS = T // NS           # i-spans == chunks
    Exp = mybir.ActivationFunctionType.Exp
    expq = _get_expq()
    ec1, ec2, ec3 = _EXPQ_C

    xnp_ = ctx.enter_context(tc.tile_pool(name="xn", bufs=2))
    xtp = ctx.enter_context(tc.tile_pool(name="xt", bufs=2))
    weip = ctx.enter_context(tc.tile_pool(name="wei", bufs=6))
    rabp = ctx.enter_context(tc.tile_pool(name="rab", bufs=2))
    bcsp = ctx.enter_context(tc.tile_pool(name="bcs", bufs=2))
    posp = ctx.enter_context(tc.tile_pool(name="posb", bufs=2))
    rdp = ctx.enter_context(tc.tile_pool(name="rdram", bufs=2, space="DRAM"))
    mmp = ctx.enter_context(tc.tile_pool(name="mm", bufs=2, space="PSUM"))
    pavp = ctx.enter_context(tc.tile_pool(name="pav", bufs=2, space="PSUM"))

    def blk_tile(nm, ptag):
        pool = pavp if ptag == "pav" else mmp
        return pool.tile([P, 2, NS], F32, tag=ptag, name=nm)

    xts = {}

    # ---- QKV projection for one chunk of 512 t-columns, as filler thunks ----
    def qkv_thunks(c):
        t0 = c * NS

        def mk_tl(tl):
            def f(ptag="blk"):
                if c not in xts:
                    xts[c] = xtp.tile([P, KT, NS], BF16, tag="xt", name="xt")
                xt = xts[c]
                xn = xnp_.tile([P, C], F32, tag="xn")
                nc.sync.dma_start(out=xn, in_=x[t0 + tl * P : t0 + (tl + 1) * P, :])
                pt = blk_tile("pt", ptag)
                for kt in range(KT):
                    nc.tensor.transpose(
                        pt[:, kt // 4, (kt % 4) * P : (kt % 4 + 1) * P],
                        xn[:, kt * P : (kt + 1) * P],
                        ident,
                    )
                nc.scalar.copy(
                    xt[:, :, tl * P : (tl + 1) * P],
                    pt.rearrange("p a (b q) -> p (a b) q", b=4),
                )

            return f

        def mk_qk(dsti, m2):
            def f(ptag="blk"):
                xt = xts[c]
                dst = qrT if dsti == 0 else kcT
                pq = blk_tile("pq", ptag)
                for mh in range(2):
                    m = 2 * m2 + mh
                    for kt in range(KT):
                        nc.tensor.matmul(
                            pq[:, mh, :],
                            wqkv[:, dsti, kt, m * P : (m + 1) * P],
                            xt[:, kt, :],
                            start=(kt == 0),
                            stop=(kt == KT - 1),
                        )
                nc.vector.tensor_copy(dst[:, 2 * m2 : 2 * m2 + 2, t0 : t0 + NS], pq)

            return f

        def mk_v(vh):
            def f(ptag="blk"):
                xt = xts[c]
                pv = blk_tile("pv", ptag)
                for th2 in range(2):
                    tl = 2 * vh + th2
                    for kt in range(KT):
                        nc.tensor.matmul(
                            pv[:, th2, :],
                            xt[:, kt, tl * P : (tl + 1) * P],
                            wqkv[:, 2, kt, :],
                            start=(kt == 0),
                            stop=(kt == KT - 1),
                        )
                nc.vector.tensor_copy(
                    v_sb[:, c * 4 + 2 * vh : c * 4 + 2 * vh + 2, :, 0:D],
                    pv.rearrange("p a (h d) -> p a h d", h=HPC),
                )

            return f

        ths = [mk_tl(tl) for tl in range(4)]
        ths += [mk_qk(d, m2) for d in range(2) for m2 in range(2)]
        ths += [mk_v(vh) for vh in range(2)]
        return ths

    # ---- output projection for one span, as filler thunks ----
    def proj_thunks(s):
        def mk(m2):
            def f(ptag="blk"):
                po = blk_tile("po", ptag)
                for mh in range(2):
                    m = 2 * m2 + mh
                    for cp in range(MT):
                        nc.tensor.matmul(
                            po[:, mh, :],
                            wp_b[:, cp, m * P : (m + 1) * P],
                            attnT[:, cp, s * NS : (s + 1) * NS],
                            start=(cp == 0),
                            stop=(cp == MT - 1),
                        )
                ps = posp.tile([P, 2, NS], BF16, tag="po_sb")
                for mh in range(2):
                    nc.vector.tensor_scalar_add(
                        ps[:, mh, :],
                        po[:, mh, :],
                        bias_sb[:, 2 * m2 + mh : 2 * m2 + mh + 1],
                    )
                nc.scalar.dma_start(
                    out=outT[
                        (2 * m2) * P : (2 * m2 + 2) * P, s * NS : (s + 1) * NS
                    ].rearrange("(h p) n -> p h n", h=2),
                    in_=ps,
                )

            return f

        return [mk(m2) for m2 in range(4)]

    # ---- attention units ----
    pend = {}
    pavs = {}
    ucount = [0]

    def front(u, route):
        p_, s, jt = u
        i0 = s * NS
        off = max(0, jt * P - i0)
        ucount[0] += 1
        psc = blk_tile("psc", "pav" if ucount[0] % 2 else "blk")
        for h in range(2):
            hp = slice(64 * h, 64 * h + 64)
            nc.tensor.matmul(
                psc[:, h, off:NS],
                kcT[hp, p_, jt * P : (jt + 1) * P],
                qrT[hp, p_, i0 + off : i0 + NS],
                start=True,
                stop=True,
                tile_position=(64 * h, 0),
            )
        wei = weip.tile([P, 2, NS], BF16, tag="wei")
        if route == "A":
            nc.scalar.activation(wei[:, :, off:NS], psc[:, :, off:NS], Exp, scale=SCALE)
        else:
            nc.vector._custom_dve(
                expq,
                out=wei[:, :, off:NS],
                in0=psc[:, :, off:NS],
                s0=ec1,
                s1=ec2,
                imm2=ec3,
            )
        if jt * P >= i0:  # tile containing the diagonal
            nc.gpsimd.affine_select(
                out=wei[:, :, off : off + P],
                in_=wei[:, :, off : off + P],
                pattern=[[0, 2], [1, P]],
                base=0,
                channel_multiplier=-1,
                compare_op=mybir.AluOpType.is_ge,
                fill=0.0,
            )
        pend[u] = wei

    def back(u):
        p_, s, jt = u
        i0 = s * NS
        jmax = (s + 1) * (NS // P)
        off = max(0, jt * P - i0)
        first, last = jt == 0, jt == jmax - 1
        wei = pend.pop(u)
        if first:
            pavs[(p_, s)] = pavp.tile([D + 1, 2, NS], F32, tag="pav", name="pav")
        pav = pavs[(p_, s)]
        for h in range(2):
            nc.tensor.matmul(
                pav[:, h, off:NS],
                v_sb[:, jt, 2 * p_ + h, :],
                wei[:, h, off:NS],
                start=first,
                stop=last,
            )
        if not last:
            return
        pav = pavs.pop((p_, s))
        rab = rabp.tile([D + 1, 2, NS], F32, tag="rab")
        # NOTE: custom DVE ops mis-execute at nonzero partition base on HW,
        # so run the recip over partitions 0..64 (base 0); rows 0..63 are
        # junk (1/attn-partials) and unused — only row 64 (1/denominator)
        # feeds the broadcast.
        nc.vector.reciprocal_approx_fast(out=rab, in_=pav)
        rd = rdp.tile([1, 2, NS], F32, tag="rd")
        nc.sync.dma_start(out=rd[0], in_=rab[D : D + 1, :, :])
        bcs = bcsp.tile([D, 2, NS], F32, tag="bcs")
        nc.sync.dma_start(out=bcs, in_=rd.to_broadcast([D, 2, NS]))
        for h in range(2):
            nc.vector.tensor_mul(
                attnT[64 * h : 64 * h + 64, p_, i0 : i0 + NS],
                pav[0:D, h, :],
                bcs[:, h, :],
            )

    # ---- the single fused stream ----
    for th in qkv_thunks(0):
        th()

    if 2 not in phases:  # decomposition benching: QKV only
        for c in range(1, TS):
            for th in qkv_thunks(c):
                th()
        return

    do3 = 3 in phases
    pat = EXP_PATTERN
    pi = 0
    span_fillers = {
        0: qkv_thunks(1),
        1: qkv_thunks(2) + (proj_thunks(0) if do3 else []),
        2: qkv_thunks(3) + (proj_thunks(1) if do3 else []),
        3: proj_thunks(2) if do3 else [],
    }
    for s in range(TS):
        units = [(p_, s, jt) for p_ in range(MT) for jt in range((s + 1) * (NS // P))]
        F = span_fillers.get(s, [])
        U = len(units)
        nf = 0
        for idx in range(U + DEPTH):
            if idx < U:
                front(units[idx], pat[pi % len(pat)])
                pi += 1
            if idx >= DEPTH:
                back(units[idx - DEPTH])
            want = ((idx + 1) * len(F)) // (U + DEPTH)
            while nf < want:
                F[nf]()
                nf += 1
        while nf < len(F):
            F[nf]()
            nf += 1
    if do3:
        for th in proj_thunks(TS - 1):
            th()

    if "dbg_qrT" in aps:
        nc.sync.dma_start(out=aps["dbg_qrT"], in_=qrT)
        nc.sync.dma_start(out=aps["dbg_kcT"], in_=kcT)
        nc.sync.dma_start(out=aps["dbg_v"], in_=v_sb)
        nc.sync.dma_start(out=aps["dbg_attnT"], in_=attnT)


def build(T=T_FULL, reps=1, phases=(1, 2, 3), debug_dumps=False):
    from contextlib import ExitStack

    nc = bacc.Bacc(
        "TRN2", target_bir_lowering=False, debug=False, num_devices=NCORES
    )
    aps = {
        "x": nc.dram_tensor("x", [T, C], F32, kind="ExternalInput").ap(),
        "wr": nc.dram_tensor("wr", [C, CH], F32, kind="ExternalInput").ap(),
        "wc": nc.dram_tensor("wc", [C, CH], F32, kind="ExternalInput").ap(),
        "wv": nc.dram_tensor("wv", [C, CH], F32, kind="ExternalInput").ap(),
        "wp": nc.dram_tensor("wp", [CH, C], F32, kind="ExternalInput").ap(),
        "bias": nc.dram_tensor("bias", [P, C // P], F32, kind="ExternalInput").ap(),
        "outT": nc.dram_tensor("outT", [C, T], BF16, kind="ExternalOutput").ap(),
    }
    if debug_dumps:
        aps["dbg_qrT"] = nc.dram_tensor(
            "dbg_qrT", [P, MT, T], BF16, kind="ExternalOutput"
        ).ap()
        aps["dbg_kcT"] = nc.dram_tensor(
            "dbg_kcT", [P, MT, T], BF16, kind="ExternalOutput"
        ).ap()
        aps["dbg_v"] = nc.dram_tensor(
            "dbg_v", [P, T // P, HPC, D + 1], BF16, kind="ExternalOutput"
        ).ap()
        aps["dbg_attnT"] = nc.dram_tensor(
            "dbg_attnT", [P, MT, T], BF16, kind="ExternalOutput"
        ).ap()
    with tile.TileContext(nc) as tc:
        with ExitStack() as ctx:
            pre = _emit_prelude(ctx, tc, aps, T)
            if reps == 1:
                _emit(ctx, tc, aps, T, pre, phases)
            else:
                with tc.For_i(
                    0,
                    reps,
                    1,
                    staggered_reset=True,
                    hint_engines=(
                        mybir.EngineType.PE,
                        mybir.EngineType.DVE,
                        mybir.EngineType.Activation,
                        mybir.EngineType.Pool,
                        mybir.EngineType.SP,
                    ),
                ):
                    _emit(ctx, tc, aps, T, pre, phases)
    nc.compile()
    return nc


def make_in_maps(x, Wk, Wq, Wv, Wproj, bproj):
    """Shard full inputs into 8 per-core input maps."""
    in_maps = []
    for c in range(NCORES):
        b, g = c // 2, c % 2
        cols = slice(CH * g, CH * (g + 1))
        in_maps.append(
            {
                "x": np.ascontiguousarray(np.asarray(x)[b], dtype=np.float32),
                "wr": np.ascontiguousarray(np.asarray(Wk)[:, cols], dtype=np.float32),
                "wc": np.ascontiguousarray(np.asarray(Wq)[:, cols], dtype=np.float32),
                "wv": np.ascontiguousarray(np.asarray(Wv)[:, cols], dtype=np.float32),
                "wp": np.ascontiguousarray(np.asarray(Wproj)[cols, :], dtype=np.float32),
                "bias": np.ascontiguousarray(
                    (0.5 * np.asarray(bproj)).reshape(C // P, P).T, dtype=np.float32
                ),
            }
        )
    return in_maps


_CACHE = {}


def kernel(x, Wk, Wq, Wv, Wproj, bproj):
    x = np.asarray(x, dtype=np.float32)
    if "nc" not in _CACHE:
        _CACHE["nc"] = build(T=x.shape[1])
    nc = _CACHE["nc"]
    in_maps = make_in_maps(x, Wk, Wq, Wv, Wproj, bproj)
    res = run_bass_kernel_spmd(nc, in_maps, list(range(NCORES)))
    out = np.empty((x.shape[0], x.shape[1], C), dtype=np.float32)
    for b in range(x.shape[0]):
        a0 = np.asarray(res.results[2 * b]["outT"], dtype=np.float32)
        a1 = np.asarray(res.results[2 * b + 1]["outT"], dtype=np.float32)
        out[b] = (a0 + a1).T
    return out


# revision 29
# speedup vs baseline: 1.0142x; 1.0142x over previous
"""Trainium2 Bass kernel: causal multi-head attention (dense transformer block).

Reference semantics (quirk: scores = K @ Q^T, scaled by C**-0.5):
    k = x @ Wk ; q = x @ Wq ; v = x @ Wv          (per-head split, H=16, D=64)
    wei[i, j] = (k_i . q_j) * C**-0.5,  masked j <= i, softmax over j
    out = (wei @ v) @ Wproj + bproj

Sharding (8 cores): data-parallel over batch (4) x tensor-parallel over
head-halves (2).  Core c handles batch c//2, heads [8*(c%2), 8*(c%2)+8).
Each core returns the transposed partial projection output in bf16;
the host sums the two partials per batch and transposes back in fp32.

v2 design (fused single-stream pipeline):
  - Whole kernel is ONE stream: QKV projection chunks, attention spans and
    the output projection are interleaved so PE / ACT / DVE / GPSIMD all
    stay busy.  QKV chunk c+1 and output-proj span s-1 are emitted as
    "filler" thunks woven between attention units of span s.
  - All activations bf16 (qrT/kcT/v/wei/attnT, weights cast once per rep);
    matmuls run bf16 at full PE rate; x is transposed on PE in fp32 and
    cast to bf16 in the PSUM->SBUF copy.
  - exp is split between ScalarE (activation Exp) and a custom 8-stage DVE
    op EXPQ: exp(s*SCALE) ~= ((1 + s(c1 + s(c2 + s*c3)))^2)^2, a degree-3
    minimax fit squared twice (always positive, exact at 0, ~1e-3 rel err
    over the +-64 score range).  Ratio set by EXP_PATTERN.
  - softmax denominator rides as a ones-column in V (AV output row 64);
    1/d via DVE reciprocal_approx_fast, broadcast across partitions by a
    DRAM-bounce DMA, folded in with a DVE multiply.
  - PSUM budget (8 banks): shared "blk" pool (scores / QKV / transposes /
    out-proj, 2 x [128,2,512] slots = 4 banks) + pav pool (AV accumulate,
    2 x [65,2,512] = 4 banks).
"""

import numpy as np

import concourse.bass as bass  # noqa: F401
import concourse.tile as tile
from concourse import bacc, mybir
from concourse.bass_utils import run_bass_kernel_spmd
from concourse.masks import make_identity

F32 = mybir.dt.float32
BF16 = mybir.dt.bfloat16

B, T_FULL, C = 4, 2048, 1024
H, NCORES, P, NS, D = 16, 8, 128, 512, 64
CH = C // 2            # per-core channels (8 heads)
HPC = CH // D          # heads per core
MT = CH // P           # head-pair tiles
KT = C // P            # k-tiles of the model dim
SCALE = 1.0 / float(np.sqrt(C))
DEPTH = 2

# exp engine per attention unit: 'A' = ScalarE, 'D' = custom DVE poly.
EXP_PATTERN = "AAD"


def _fit_expq(smax=64.0):
    """Fit p(s) = 1 + c1 s + c2 s^2 + c3 s^3 so that p^4 ~= exp(SCALE*s)
    on [-smax, smax], minimizing max relative error (Lawson iteration)."""
    s = np.linspace(-smax, smax, 4001)
    t = np.exp(SCALE * s / 4.0)
    A = np.stack([s, s * s, s**3], 1)
    w = np.ones_like(s)
    c = np.zeros(3)
    for _ in range(80):
        Wt = (w / t)[:, None]
        c, *_ = np.linalg.lstsq(A * Wt, (t - 1.0) * Wt[:, 0], rcond=None)
        r = np.abs((1.0 + A @ c) / t - 1.0)
        w = w * np.maximum(r, 1e-9) ** 0.7
        w /= w.mean()
    g = (1.0 + A @ c) ** 4
    rel = np.abs(g / np.exp(SCALE * s) - 1.0)
    return [float(v) for v in c], float(rel.max())


_EXPQ_C, _EXPQ_MAXERR = _fit_expq()
_EXPQ_OP = None


def _get_expq():
    global _EXPQ_OP
    if _EXPQ_OP is None:
        from concourse.dve_spec import Spec, Src0, C0, C1, C2, One, sq, lower
        from concourse.dve_uop import DveOpSpec
        from concourse.dve_ops import DveOp, OPS

        name = "EXPQ_MHA_V1"
        body = sq(sq(One + Src0 * (C0 + Src0 * (C1 + Src0 * C2))))

        def _ref(in0, in1, s0, s1, imm2):
            p = 1.0 + in0 * (s0 + in0 * (s1 + in0 * imm2))
            p = p * p
            return p * p

        spec = Spec(body=body, reference=_ref)
        shas = {
            ver: DveOpSpec(
                name=name, opcode=0, uops=lower(spec, ver=ver), rd1_en=False
            ).sha(ver)
            for ver in ("v3", "v4")
        }
        op = DveOp(name, spec, subdim=False, uops_sha=shas)
        for i, o in enumerate(OPS):
            if o.name == name:
                OPS[i] = op
                break
        else:
            OPS.append(op)
        from concourse import dve_ops as _dops

        _dops.CUSTOM_DVE_SPECS[name] = spec
        _dops._SUB_OPCODE_FOR_NAME[name] = _dops._CUSTOM_DVE_ROW_BASE + next(
            i for i, o in enumerate(OPS) if o.name == name
        )
        assert max(_dops._SUB_OPCODE_FOR_NAME.values()) < 0x20
        _EXPQ_OP = op
    return _EXPQ_OP


def _emit_prelude(ctx, tc, aps, T):
    """Once-per-kernel setup: constants, weight loads + bf16 casts, and the
    persistent SBUF tiles.  Kept outside the reps loop."""
    nc = tc.nc
    wr, wc, wv, wp, bias_d = (aps[k] for k in ("wr", "wc", "wv", "wp", "bias"))
    TTL = T // P

    consts = ctx.enter_context(tc.tile_pool(name="consts", bufs=1))
    ident = consts.tile([P, P], F32)
    make_identity(nc, ident)
    onesf = consts.tile([P, P], F32)
    nc.vector.memset(onesf, 1.0)
    bias_sb = consts.tile([P, C // P], F32)
    nc.sync.dma_start(out=bias_sb, in_=bias_d)

    acts = ctx.enter_context(tc.tile_pool(name="acts", bufs=1))
    qrT = acts.tile([P, MT, T], BF16)      # row operand (x@Wk), head-transposed
    kcT = acts.tile([P, MT, T], BF16)      # col operand (x@Wq), head-transposed
    v_sb = acts.tile([P, TTL, HPC, D + 1], BF16)  # V + ones column per head
    attnT = acts.tile([P, MT, T], BF16)
    wqkv = acts.tile([P, 3, KT, CH], BF16)
    wp_b = acts.tile([P, MT, C], BF16)

    nc.vector.tensor_copy(
        v_sb[:, :, :, D : D + 1],
        onesf[:, 0 : TTL * HPC].rearrange("p (a b c) -> p a b c", a=TTL, b=HPC, c=1),
    )

    with tc.tile_pool(name="stage", bufs=2) as stage:
        for wi, wd in enumerate((wr, wc, wv)):
            wre = wd.rearrange("(kt p) ch -> p kt ch", p=P)
            for hh in range(2):
                st = stage.tile([P, KT // 2, CH], F32, tag="st")
                nc.gpsimd.dma_start(out=st, in_=wre[:, 4 * hh : 4 * hh + 4, :])
                nc.scalar.copy(wqkv[:, wi, 4 * hh : 4 * hh + 4, :], st)
        wpre = wp.rearrange("(cp p) c -> p cp c", p=P)
        for cp in range(MT):
            st = stage.tile([P, C], F32, tag="st")
            nc.gpsimd.dma_start(out=st, in_=wpre[:, cp, :])
            nc.scalar.copy(wp_b[:, cp, :], st)

    return dict(
        ident=ident, bias_sb=bias_sb, qrT=qrT, kcT=kcT, v_sb=v_sb,
        attnT=attnT, wqkv=wqkv, wp_b=wp_b,
    )


def _emit(ctx, tc, aps, T, pre, phases=(1, 2, 3)):
    nc = tc.nc
    x, outT = aps["x"], aps["outT"]
    ident, bias_sb = pre["ident"], pre["bias_sb"]
    qrT, kcT, v_sb, attnT = pre["qrT"], pre["kcT"], pre["v_sb"], pre["attnT"]
    wqkv, wp_b = pre["wqkv"], pre["wp_b"]
    TS = T // NS           # i-spans == chunks
    Exp = mybir.ActivationFunctionType.Exp
    expq = _get_expq()
    ec1, ec2, ec3 = _EXPQ_C

    xnp_ = ctx.enter_context(tc.tile_pool(name="xn", bufs=2))
    xtp = ctx.enter_context(tc.tile_pool(name="xt", bufs=2))
    weip = ctx.enter_context(tc.tile_pool(name="wei", bufs=6))
    rabp = ctx.enter_context(tc.tile_pool(name="rab", bufs=2))
    bcsp = ctx.enter_context(tc.tile_pool(name="bcs", bufs=2))
    posp = ctx.enter_context(tc.tile_pool(name="posb", bufs=2))
    rdp = ctx.enter_context(tc.tile_pool(name="rdram", bufs=2, space="DRAM"))
    mmp = ctx.enter_context(tc.tile_pool(name="mm", bufs=2, space="PSUM"))
    pavp = ctx.enter_context(tc.tile_pool(name="pav", bufs=2, space="PSUM"))

    def blk_tile(nm, ptag):
        pool = pavp if ptag == "pav" else mmp
        return pool.tile([P, 2, NS], F32, tag=ptag, name=nm)

    xts = {}

    # ---- QKV projection for one chunk of 512 t-columns, as filler thunks ----
    def qkv_thunks(c):
        t0 = c * NS

        def mk_tl(tl):
            def f(ptag="blk"):
                if c not in xts:
                    xts[c] = xtp.tile([P, KT, NS], BF16, tag="xt", name="xt")
                xt = xts[c]
                xn = xnp_.tile([P, C], F32, tag="xn")
                nc.sync.dma_start(out=xn, in_=x[t0 + tl * P : t0 + (tl + 1) * P, :])
                pt = blk_tile("pt", ptag)
                for kt in range(KT):
                    nc.tensor.transpose(
                        pt[:, kt // 4, (kt % 4) * P : (kt % 4 + 1) * P],
                        xn[:, kt * P : (kt + 1) * P],
                        ident,
                    )
                nc.scalar.copy(
                    xt[:, :, tl * P : (tl + 1) * P],
                    pt.rearrange("p a (b q) -> p (a b) q", b=4),
                )

            return f

        def mk_qk(dsti, m2):
            def f(ptag="blk"):
                xt = xts[c]
                dst = qrT if dsti == 0 else kcT
                pq = blk_tile("pq", ptag)
                for mh in range(2):
                    m = 2 * m2 + mh
                    for kt in range(KT):
                        nc.tensor.matmul(
                            pq[:, mh, :],
                            wqkv[:, dsti, kt, m * P : (m + 1) * P],
                            xt[:, kt, :],
                            start=(kt == 0),
                            stop=(kt == KT - 1),
                        )
                nc.vector.tensor_copy(dst[:, 2 * m2 : 2 * m2 + 2, t0 : t0 + NS], pq)

            return f

        def mk_v(vh):
            def f(ptag="blk"):
                xt = xts[c]
                pv = blk_tile("pv", ptag)
                for th2 in range(2):
                    tl = 2 * vh + th2
                    for kt in range(KT):
                        nc.tensor.matmul(
                            pv[:, th2, :],
                            xt[:, kt, tl * P : (tl + 1) * P],
                            wqkv[:, 2, kt, :],
                            start=(kt == 0),
                            stop=(kt == KT - 1),
                        )
                nc.vector.tensor_copy(
                    v_sb[:, c * 4 + 2 * vh : c * 4 + 2 * vh + 2, :, 0:D],
                    pv.rearrange("p a (h d) -> p a h d", h=HPC),
                )

            return f

        ths = [mk_tl(tl) for tl in range(4)]
        ths += [mk_qk(d, m2) for d in range(2) for m2 in range(2)]
        ths += [mk_v(vh) for vh in range(2)]
        return ths

    # ---- output projection for one span, as filler thunks ----
    def proj_thunks(s):
        def mk(m2):
            def f(ptag="blk"):
                po = blk_tile("po", ptag)
                for mh in range(2):
                    m = 2 * m2 + mh
                    for cp in range(MT):
                        nc.tensor.matmul(
                            po[:, mh, :],
                            wp_b[:, cp, m * P : (m + 1) * P],
                            attnT[:, cp, s * NS : (s + 1) * NS],
                            start=(cp == 0),
                            stop=(cp == MT - 1),
                        )
                ps = posp.tile([P, 2, NS], BF16, tag="po_sb")
                for mh in range(2):
                    nc.vector.tensor_scalar_add(
                        ps[:, mh, :],
                        po[:, mh, :],
                        bias_sb[:, 2 * m2 + mh : 2 * m2 + mh + 1],
                    )
                nc.scalar.dma_start(
                    out=outT[
                        (2 * m2) * P : (2 * m2 + 2) * P, s * NS : (s + 1) * NS
                    ].rearrange("(h p) n -> p h n", h=2),
                    in_=ps,
                )

            return f

        return [mk(m2) for m2 in range(4)]

    # ---- attention units ----
    pend = {}
    pavs = {}
    ucount = [0]

    def front(u, route):
        p_, s, jt = u
        i0 = s * NS
        off = max(0, jt * P - i0)
        ucount[0] += 1
        psc = blk_tile("psc", "pav" if ucount[0] % 2 else "blk")
        for h in range(2):
            hp = slice(64 * h, 64 * h + 64)
            nc.tensor.matmul(
                psc[:, h, off:NS],
                kcT[hp, p_, jt * P : (jt + 1) * P],
                qrT[hp, p_, i0 + off : i0 + NS],
                start=True,
                stop=True,
                tile_position=(64 * h, 0),
            )
        wei = weip.tile([P, 2, NS], BF16, tag="wei")
        if route == "A":
            nc.scalar.activation(wei[:, :, off:NS], psc[:, :, off:NS], Exp, scale=SCALE)
        else:
            nc.vector._custom_dve(
                expq,
                out=wei[:, :, off:NS],
                in0=psc[:, :, off:NS],
                s0=ec1,
                s1=ec2,
                imm2=ec3,
            )
        if jt * P >= i0:  # tile containing the diagonal
            nc.gpsimd.affine_select(
                out=wei[:, :, off : off + P],
                in_=wei[:, :, off : off + P],
                pattern=[[0, 2], [1, P]],
                base=0,
                channel_multiplier=-1,
                compare_op=mybir.AluOpType.is_ge,
                fill=0.0,
            )
        pend[u] = wei

    def back(u):
        p_, s, jt = u
        i0 = s * NS
        jmax = (s + 1) * (NS // P)
        off = max(0, jt * P - i0)
        first, last = jt == 0, jt == jmax - 1
        wei = pend.pop(u)
        if first:
            pavs[(p_, s)] = pavp.tile([D + 1, 2, NS], F32, tag="pav", name="pav")
        pav = pavs[(p_, s)]
        for h in range(2):
            nc.tensor.matmul(
                pav[:, h, off:NS],
                v_sb[:, jt, 2 * p_ + h, :],
                wei[:, h, off:NS],
                start=first,
                stop=last,
            )
        if not last:
            return
        pav = pavs.pop((p_, s))
        rab = rabp.tile([D + 1, 2, NS], F32, tag="rab")
        # NOTE: custom DVE ops mis-execute at nonzero partition base on HW,
        # so run the recip over partitions 0..64 (base 0); rows 0..63 are
        # junk (1/attn-partials) and unused — only row 64 (1/denominator)
        # feeds the broadcast.
        nc.vector.reciprocal_approx_fast(out=rab, in_=pav)
        rd = rdp.tile([1, 2, NS], F32, tag="rd")
        nc.sync.dma_start(out=rd[0], in_=rab[D : D + 1, :, :])
        bcs = bcsp.tile([D, 2, NS], F32, tag="bcs")
        nc.sync.dma_start(out=bcs, in_=rd.to_broadcast([D, 2, NS]))
        for h in range(2):
            nc.vector.tensor_mul(
                attnT[64 * h : 64 * h + 64, p_, i0 : i0 + NS],
                pav[0:D, h, :],
                bcs[:, h, :],
            )

    # ---- the single fused stream ----
    for th in qkv_thunks(0):
        th()

    if 2 not in phases:  # decomposition benching: QKV only
        for c in range(1, TS):
            for th in qkv_thunks(c):
                th()
        return

    do3 = 3 in phases
    pat = EXP_PATTERN
    pi = 0
    span_fillers = {
        0: qkv_thunks(1),
        1: qkv_thunks(2) + (proj_thunks(0) if do3 else []),
        2: qkv_thunks(3) + (proj_thunks(1) if do3 else []),
        3: proj_thunks(2) if do3 else [],
    }
    for s in range(TS):
        units = [(p_, s, jt) for p_ in range(MT) for jt in range((s + 1) * (NS // P))]
        F = span_fillers.get(s, [])
        U = len(units)
        nf = 0
        for idx in range(U + DEPTH):
            if idx < U:
                front(units[idx], pat[pi % len(pat)])
                pi += 1
            if idx >= DEPTH:
                back(units[idx - DEPTH])
            want = ((idx + 1) * len(F)) // (U + DEPTH)
            while nf < want:
                F[nf]()
                nf += 1
        while nf < len(F):
            F[nf]()
            nf += 1
    if do3:
        for th in proj_thunks(TS - 1):
            th()

    if "dbg_qrT" in aps:
        nc.sync.dma_start(out=aps["dbg_qrT"], in_=qrT)
        nc.sync.dma_start(out=aps["dbg_kcT"], in_=kcT)
        nc.sync.dma_start(out=aps["dbg_v"], in_=v_sb)
        nc.sync.dma_start(out=aps["dbg_attnT"], in_=attnT)


def build(T=T_FULL, reps=1, phases=(1, 2, 3), debug_dumps=False):
    from contextlib import ExitStack

    nc = bacc.Bacc(
        "TRN2", target_bir_lowering=False, debug=False, num_devices=NCORES
    )
    aps = {
        "x": nc.dram_tensor("x", [T, C], F32, kind="ExternalInput").ap(),
        "wr": nc.dram_tensor("wr", [C, CH], F32, kind="ExternalInput").ap(),
        "wc": nc.dram_tensor("wc", [C, CH], F32, kind="ExternalInput").ap(),
        "wv": nc.dram_tensor("wv", [C, CH], F32, kind="ExternalInput").ap(),
        "wp": nc.dram_tensor("wp", [CH, C], F32, kind="ExternalInput").ap(),
        "bias": nc.dram_tensor("bias", [P, C // P], F32, kind="ExternalInput").ap(),
        "outT": nc.dram_tensor("outT", [C, T], BF16, kind="ExternalOutput").ap(),
    }
    if debug_dumps:
        aps["dbg_qrT"] = nc.dram_tensor(
            "dbg_qrT", [P, MT, T], BF16, kind="ExternalOutput"
        ).ap()
        aps["dbg_kcT"] = nc.dram_tensor(
            "dbg_kcT", [P, MT, T], BF16, kind="ExternalOutput"
        ).ap()
        aps["dbg_v"] = nc.dram_tensor(
            "dbg_v", [P, T // P, HPC, D + 1], BF16, kind="ExternalOutput"
        ).ap()
        aps["dbg_attnT"] = nc.dram_tensor(
            "dbg_attnT", [P, MT, T], BF16, kind="ExternalOutput"
        ).ap()
    with tile.TileContext(nc) as tc:
        with ExitStack() as ctx:
            pre = _emit_prelude(ctx, tc, aps, T)
            if reps == 1:
                _emit(ctx, tc, aps, T, pre, phases)
            else:
                with tc.For_i(
                    0,
                    reps,
                    1,
                    hint_engines=(
                        mybir.EngineType.PE,
                        mybir.EngineType.DVE,
                        mybir.EngineType.Activation,
                        mybir.EngineType.Pool,
                        mybir.EngineType.SP,
                    ),
                ):
                    _emit(ctx, tc, aps, T, pre, phases)
    nc.compile()
    return nc


def make_in_maps(x, Wk, Wq, Wv, Wproj, bproj):
    """Shard full inputs into 8 per-core input maps."""
    in_maps = []
    for c in range(NCORES):
        b, g = c // 2, c % 2
        cols = slice(CH * g, CH * (g + 1))
        in_maps.append(
            {
                "x": np.ascontiguousarray(np.asarray(x)[b], dtype=np.float32),
                "wr": np.ascontiguousarray(np.asarray(Wk)[:, cols], dtype=np.float32),
                "wc": np.ascontiguousarray(np.asarray(Wq)[:, cols], dtype=np.float32),
                "wv": np.ascontiguousarray(np.asarray(Wv)[:, cols], dtype=np.float32),
                "wp": np.ascontiguousarray(np.asarray(Wproj)[cols, :], dtype=np.float32),
                "bias": np.ascontiguousarray(
                    (0.5 * np.asarray(bproj)).reshape(C // P, P).T, dtype=np.float32
                ),
            }
        )
    return in_maps


_CACHE = {}


def kernel(x, Wk, Wq, Wv, Wproj, bproj):
    x = np.asarray(x, dtype=np.float32)
    if "nc" not in _CACHE:
        _CACHE["nc"] = build(T=x.shape[1])
    nc = _CACHE["nc"]
    in_maps = make_in_maps(x, Wk, Wq, Wv, Wproj, bproj)
    res = run_bass_kernel_spmd(nc, in_maps, list(range(NCORES)))
    out = np.empty((x.shape[0], x.shape[1], C), dtype=np.float32)
    for b in range(x.shape[0]):
        a0 = np.asarray(res.results[2 * b]["outT"], dtype=np.float32)
        a1 = np.asarray(res.results[2 * b + 1]["outT"], dtype=np.float32)
        out[b] = (a0 + a1).T
    return out
